# revision 6
# baseline (speedup 1.0000x reference)
"""Trainium2 Bass kernel for nn_BlockFast (MoE routing block).

Computation (per token n):
  xa = x @ P_w.T                       [N, 64]
  z_j = xa @ U_j.T, top-3 -> softmax -> dense combine C_j [N, 12], j=1..3
  h = gelu(sum_e C1[:,e] * (x @ W1[e].T))
  y = sum_e C2[:,e] * (h @ W2[e].T) + C3 @ b2

Strategy: data-parallel over tokens on 8 cores (12-expert tapes replicated).
Dense-over-experts compute with f32r (relaxed fp32) matmuls on the PE at
1 cyc/row; per-token combine weights applied as fused per-partition
scalar_tensor_tensor epilogues with tokens on PSUM partitions.
"""

import numpy as np

import concourse.bass as bass
import concourse.bacc as bacc
import concourse.mybir as mybir
import concourse.tile as tile
from concourse.bass_utils import run_bass_kernel_spmd
from concourse.masks import make_identity

F32 = mybir.dt.float32
F32R = mybir.dt.float32r

N_CORES = 8
D = 1024        # D_in == H == D_out
L = 12          # experts per tape
D_ADDR = 64
KS = D // 128   # contraction subtiles
TAU_SCALE = float(1.0 / np.float32(1.0 + 1e-8))  # 1/(tau+eps) as f32
NEG_BIG = -1e30


def build(t_c: int, st_tok: int = 1024, gelu: bool = True):
    """Build the per-core kernel for t_c tokens, st_tok tokens per supertile.

    gelu=False replaces the Gelu with Identity (CoreSim lacks a Gelu impl)."""
    assert t_c % st_tok == 0 and st_tok % 128 == 0
    gelu_fn = (mybir.ActivationFunctionType.Gelu if gelu
               else mybir.ActivationFunctionType.Identity)
    n_st = t_c // st_tok
    n_tt = st_tok // 128  # 128-token tiles per supertile

    nc = bacc.Bacc(None, target_bir_lowering=False)

    xt = nc.dram_tensor("xt", [D, t_c], F32, kind="ExternalInput")
    ucat_t = nc.dram_tensor("ucat_t", [D_ADDR, 3 * L], F32, kind="ExternalInput")
    p_w = nc.dram_tensor("p_w", [D_ADDR, D], F32, kind="ExternalInput")
    w1t = nc.dram_tensor("w1t", [L, D, D], F32, kind="ExternalInput")
    w2t = nc.dram_tensor("w2t", [L, D, D], F32, kind="ExternalInput")
    b2 = nc.dram_tensor("b2", [L, D], F32, kind="ExternalInput")
    y = nc.dram_tensor("y", [t_c, D], F32, kind="ExternalOutput")

    with tile.TileContext(nc) as tc:
        with (
            tc.tile_pool(name="consts", bufs=1) as consts,
            tc.tile_pool(name="xpool", bufs=1) as xpool,
            tc.tile_pool(name="wpool", bufs=2) as wpool,
            tc.tile_pool(name="hpool", bufs=1) as hpool,
            tc.tile_pool(name="cpool", bufs=1) as cpool,
            tc.tile_pool(name="rt", bufs=2) as rt,
            tc.tile_pool(name="ps_mm", bufs=4, space="PSUM") as ps_mm,
            tc.tile_pool(name="ps_tr", bufs=2, space="PSUM") as ps_tr,
            tc.tile_pool(name="ps_sm", bufs=2, space="PSUM") as ps_sm,
        ):
            # ---- one-time prep -------------------------------------------
            ident = consts.tile([128, 128], F32)
            make_identity(nc, ident[:])
            b2_sb = consts.tile([L, D], F32)
            nc.sync.dma_start(b2_sb[:], b2[:])

            # A = Ucat @ P_w : [36, D]; need AT [128, KS, 36]
            u_sb = consts.tile([D_ADDR, 3 * L], F32)
            pw_sb = consts.tile([D_ADDR, D], F32)
            nc.sync.dma_start(u_sb[:], ucat_t[:])
            nc.sync.dma_start(pw_sb[:], p_w[:])
            a_sb = consts.tile([3 * L, D], F32)
            for half in range(2):
                ps_a = ps_mm.tile([3 * L, 512], F32, name="ps_a", tag="mm")
                nc.tensor.matmul(ps_a[:], u_sb[:], pw_sb[:, half * 512:(half + 1) * 512],
                                 start=True, stop=True)
                nc.vector.tensor_copy(a_sb[:, half * 512:(half + 1) * 512], ps_a[:])
            at_sb = consts.tile([128, KS, 3 * L], F32)
            for ks in range(KS):
                ps_at = ps_tr.tile([128, 3 * L], F32, name="ps_at", tag="tr")
                nc.tensor.transpose(ps_at[:], a_sb[:, ks * 128:(ks + 1) * 128],
                                    ident[:3 * L, :3 * L])
                nc.vector.tensor_copy(at_sb[:, ks, :], ps_at[:])

            # ---- persistent per-supertile tiles --------------------------
            for st in range(n_st):
                t0 = st * st_tok
                # x^T supertile [128, KS, st_tok], f32r view for the big GEMMs
                xt_sb = xpool.tile([128, KS, st_tok], F32R, name="xt_sb")
                nc.sync.dma_start(
                    xt_sb[:],
                    xt[:, t0:t0 + st_tok]
                    .rearrange("(ks kp) t -> kp ks t", kp=128)
                    .bitcast(F32R),
                )
                xt_f32 = xt_sb[:].bitcast(F32)

                # ---- routing: z = x @ A.T, then dense top-3 combine ------
                z_sb = cpool.tile([128, n_tt, 3 * L], F32, name="z_sb")
                for t in range(n_tt):
                    ps_z = ps_sm.tile([128, 3 * L], F32, name="ps_z", tag="sm")
                    for ks in range(KS):
                        nc.tensor.matmul(
                            ps_z[:],
                            xt_f32[:, ks, t * 128:(t + 1) * 128],
                            at_sb[:, ks, :],
                            start=(ks == 0), stop=(ks == KS - 1),
                        )
                    nc.vector.tensor_copy(z_sb[:, t, :], ps_z[:])

                c_sb = cpool.tile([128, n_tt, 3 * L], F32, name="c_sb")
                sh12 = [128, n_tt, L]
                for j in range(3):
                    zj = z_sb[:, :, j * L:(j + 1) * L]
                    cj = c_sb[:, :, j * L:(j + 1) * L]
                    m1 = rt.tile([128, n_tt, 1], F32, name="m1")
                    m2 = rt.tile([128, n_tt, 1], F32, name="m2")
                    m3 = rt.tile([128, n_tt, 1], F32, name="m3")
                    msk = rt.tile(sh12, F32, name="msk")
                    z1 = rt.tile(sh12, F32, name="z1")
                    z2 = rt.tile(sh12, F32, name="z2")
                    te = rt.tile(sh12, F32, name="te")
                    nc.vector.reduce_max(m1[:, :, 0], zj, axis=mybir.AxisListType.X)
                    nc.vector.tensor_tensor(msk[:], zj, m1.to_broadcast(sh12),
                                            mybir.AluOpType.is_equal)
                    nc.vector.scalar_tensor_tensor(z1[:], msk[:], NEG_BIG, zj,
                                                   mybir.AluOpType.mult,
                                                   mybir.AluOpType.add)
                    nc.vector.reduce_max(m2[:, :, 0], z1[:], axis=mybir.AxisListType.X)
                    nc.vector.tensor_tensor(msk[:], z1[:], m2.to_broadcast(sh12),
                                            mybir.AluOpType.is_equal)
                    nc.vector.scalar_tensor_tensor(z2[:], msk[:], NEG_BIG, z1[:],
                                                   mybir.AluOpType.mult,
                                                   mybir.AluOpType.add)
                    nc.vector.reduce_max(m3[:, :, 0], z2[:], axis=mybir.AxisListType.X)
                    # denom = 1 + exp((m2-m1)*s) + exp((m3-m1)*s); rec = 1/denom
                    e2 = rt.tile([128, n_tt, 1], F32, name="e2")
                    e3 = rt.tile([128, n_tt, 1], F32, name="e3")
                    den = rt.tile([128, n_tt, 1], F32, name="den")
                    rec = rt.tile([128, n_tt, 1], F32, name="rec")
                    nc.vector.tensor_sub(e2[:], m2[:], m1[:])
                    nc.vector.tensor_sub(e3[:], m3[:], m1[:])
                    nc.scalar.activation(e2[:], e2[:], mybir.ActivationFunctionType.Exp,
                                         scale=TAU_SCALE)
                    nc.scalar.activation(e3[:], e3[:], mybir.ActivationFunctionType.Exp,
                                         scale=TAU_SCALE)
                    nc.vector.tensor_add(den[:], e2[:], e3[:])
                    nc.vector.tensor_scalar_add(den[:], den[:], 1.0)
                    nc.vector.reciprocal(rec[:], den[:])
                    # C = (z >= m3) * exp((z-m1)*s) * rec
                    nc.vector.tensor_sub(te[:], zj, m1.to_broadcast(sh12))
                    nc.scalar.activation(te[:], te[:], mybir.ActivationFunctionType.Exp,
                                         scale=TAU_SCALE)
                    nc.vector.tensor_tensor(msk[:], zj, m3.to_broadcast(sh12),
                                            mybir.AluOpType.is_ge)
                    nc.vector.tensor_tensor(te[:], te[:], msk[:], mybir.AluOpType.mult)
                    nc.vector.tensor_tensor(cj, te[:], rec.to_broadcast(sh12),
                                            mybir.AluOpType.mult)

                # ---- layer 1: h = gelu(sum_e C1[:,e] * (x @ W1[e].T)) ----
                h_acc = hpool.tile([128, n_tt, D], F32, name="h_acc")
                for e in range(L):
                    for half in range(2):
                        w_sb = wpool.tile([128, KS, 512], F32R, name="w_sb", tag="w")
                        nc.sync.dma_start(
                            w_sb[:],
                            w1t[e, :, half * 512:(half + 1) * 512]
                            .rearrange("(ks kp) h -> kp ks h", kp=128)
                            .bitcast(F32R),
                        )
                        for t in range(n_tt):
                            ps = ps_mm.tile([128, 512], F32, name="ps", tag="mm")
                            for ks in range(KS):
                                nc.tensor.matmul(
                                    ps[:],
                                    xt_sb[:, ks, t * 128:(t + 1) * 128],
                                    w_sb[:, ks, :],
                                    start=(ks == 0), stop=(ks == KS - 1),
                                )
                            hs = h_acc[:, t, half * 512:(half + 1) * 512]
                            ce = c_sb[:, t, e:e + 1]
                            if e == 0:
                                nc.vector.tensor_scalar(hs, ps[:], ce, None,
                                                        mybir.AluOpType.mult)
                            else:
                                nc.vector.scalar_tensor_tensor(
                                    hs, ps[:], ce, hs,
                                    mybir.AluOpType.mult, mybir.AluOpType.add)

                # gelu in place (exact erf-based LUT)
                for t in range(n_tt):
                    nc.scalar.activation(h_acc[:, t, :], h_acc[:, t, :], gelu_fn)

                # ---- transpose h -> ht [128, KS, st_tok] f32r ------------
                ht_sb = hpool.tile([128, KS, st_tok], F32R, name="ht_sb")
                for t in range(n_tt):
                    for hs in range(KS):
                        ps_t = ps_tr.tile([128, 128], F32, name="ps_t", tag="tr")
                        nc.tensor.transpose(
                            ps_t[:], h_acc[:, t, hs * 128:(hs + 1) * 128], ident[:])
                        nc.vector.tensor_copy(ht_sb[:, hs, t * 128:(t + 1) * 128], ps_t[:])

                # ---- layer 2 init: y = C3 @ b2 ---------------------------
                y_acc = hpool.tile([128, n_tt, D], F32, name="y_acc")
                for t in range(n_tt):
                    ps_c = ps_sm.tile([L, 128], F32, name="ps_c", tag="sm")
                    nc.tensor.transpose(ps_c[:], c_sb[:, t, 2 * L:3 * L], ident[:])
                    c3t = rt.tile([L, 128], F32, name="c3t")
                    nc.vector.tensor_copy(c3t[:], ps_c[:])
                    for half in range(2):
                        ps_b = ps_mm.tile([128, 512], F32, name="ps_b", tag="mm")
                        nc.tensor.matmul(ps_b[:], c3t[:],
                                         b2_sb[:, half * 512:(half + 1) * 512],
                                         start=True, stop=True)
                        nc.vector.tensor_copy(y_acc[:, t, half * 512:(half + 1) * 512],
                                              ps_b[:])

                # ---- layer 2: y += sum_e C2[:,e] * (h @ W2[e].T) ---------
                for e in range(L):
                    for half in range(2):
                        w_sb = wpool.tile([128, KS, 512], F32R, name="w_sb", tag="w")
                        nc.sync.dma_start(
                            w_sb[:],
                            w2t[e, :, half * 512:(half + 1) * 512]
                            .rearrange("(ks kp) h -> kp ks h", kp=128)
                            .bitcast(F32R),
                        )
                        for t in range(n_tt):
                            ps = ps_mm.tile([128, 512], F32, name="ps", tag="mm")
                            for ks in range(KS):
                                nc.tensor.matmul(
                                    ps[:],
                                    ht_sb[:, ks, t * 128:(t + 1) * 128],
                                    w_sb[:, ks, :],
                                    start=(ks == 0), stop=(ks == KS - 1),
                                )
                            ys = y_acc[:, t, half * 512:(half + 1) * 512]
                            ce = c_sb[:, t, L + e:L + e + 1]
                            nc.vector.scalar_tensor_tensor(
                                ys, ps[:], ce, ys,
                                mybir.AluOpType.mult, mybir.AluOpType.add)

                # ---- store y ---------------------------------------------
                nc.sync.dma_start(
                    y[t0:t0 + st_tok, :].rearrange("(tt p) d -> p tt d", p=128),
                    y_acc[:],
                )
    nc.finalize()
    return nc


_NC_CACHE = {}


def _get_nc(t_c):
    if t_c not in _NC_CACHE:
        _NC_CACHE[t_c] = build(t_c)
    return _NC_CACHE[t_c]


def kernel(x, P_w, U1, U2, U3, W1, W2, b2):
    B, T, _ = x.shape
    n_tok = B * T
    t_c = n_tok // N_CORES
    xf = np.ascontiguousarray(x.reshape(n_tok, D))

    ucat_t = np.ascontiguousarray(np.concatenate([U1, U2, U3], axis=0).T)
    w1t = np.ascontiguousarray(W1.transpose(0, 2, 1))
    w2t = np.ascontiguousarray(W2.transpose(0, 2, 1))

    nc = _get_nc(t_c)
    in_maps = []
    for c in range(N_CORES):
        xt_c = np.ascontiguousarray(xf[c * t_c:(c + 1) * t_c].T)
        in_maps.append({
            "xt": xt_c,
            "ucat_t": ucat_t,
            "p_w": np.ascontiguousarray(P_w),
            "w1t": w1t,
            "w2t": w2t,
            "b2": np.ascontiguousarray(b2),
        })
    res = run_bass_kernel_spmd(nc, in_maps, core_ids=list(range(N_CORES)))
    y = np.concatenate([res.results[c]["y"] for c in range(N_CORES)], axis=0)
    return y.reshape(B, T, D)


# revision 15
# speedup vs baseline: 2.5170x; 2.5170x over previous
"""Trainium2 Bass kernel for nn_BlockFast (MoE routing block).

Computation (per token n):
  xa = x @ P_w.T                       [N, 64]
  z_j = xa @ U_j.T, top-3 -> softmax -> dense combine C_j [N, 12], j=1..3
  h = gelu(sum_e C1[:,e] * (x @ W1[e].T))
  y = sum_e C2[:,e] * (h @ W2[e].T) + C3 @ b2

Strategy: data-parallel over tokens on 8 cores (12-expert tapes replicated).
Dense-over-experts compute with f32r (relaxed fp32) matmuls on the PE at
1 cyc/row; per-token combine weights applied as fused per-partition
scalar_tensor_tensor epilogues with tokens on PSUM partitions.
"""

import numpy as np

import concourse.bass as bass
import concourse.bacc as bacc
import concourse.mybir as mybir
import concourse.tile as tile
from concourse.bass_utils import run_bass_kernel_spmd
from concourse.masks import make_identity

F32 = mybir.dt.float32
F32R = mybir.dt.float32r

N_CORES = 8
D = 1024        # D_in == H == D_out
L = 12          # experts per tape
D_ADDR = 64
KS = D // 128   # contraction subtiles
TAU_SCALE = float(1.0 / np.float32(1.0 + 1e-8))  # 1/(tau+eps) as f32
NEG_BIG = -1e30


def build(t_c: int, st_tok: int = 1024, gelu: bool = True):
    """Build the per-core kernel for t_c tokens, st_tok tokens per supertile.

    gelu=False replaces the Gelu with Identity (CoreSim lacks a Gelu impl)."""
    assert t_c % st_tok == 0 and st_tok % 128 == 0
    gelu_fn = (mybir.ActivationFunctionType.Gelu if gelu
               else mybir.ActivationFunctionType.Identity)
    n_st = t_c // st_tok
    n_tt = st_tok // 128  # 128-token tiles per supertile

    nc = bacc.Bacc(None, target_bir_lowering=False)

    xt = nc.dram_tensor("xt", [D, t_c], F32, kind="ExternalInput")
    ucat_t = nc.dram_tensor("ucat_t", [D_ADDR, 3 * L], F32, kind="ExternalInput")
    # P_w.T pre-tiled on host: [kp, ks, d_addr]
    p_w = nc.dram_tensor("p_w", [128, KS, D_ADDR], F32, kind="ExternalInput")
    # weights pre-arranged on host as [e, half, kp, ks, out512] so each
    # (e, half) chunk is one contiguous 2MB DMA (strided loads measured ~5x
    # slower than contiguous)
    w1t = nc.dram_tensor("w1t", [L, 2, 128, KS, 512], F32, kind="ExternalInput")
    w2t = nc.dram_tensor("w2t", [L, 2, 128, KS, 512], F32, kind="ExternalInput")
    b2 = nc.dram_tensor("b2", [L, D], F32, kind="ExternalInput")
    y = nc.dram_tensor("y", [t_c, D], F32, kind="ExternalOutput")

    with tile.TileContext(nc) as tc:
        with (
            tc.tile_pool(name="consts", bufs=1) as consts,
            tc.tile_pool(name="xpool", bufs=1) as xpool,
            tc.tile_pool(name="wpool", bufs=2) as wpool,
            tc.tile_pool(name="hpool", bufs=1) as hpool,
            tc.tile_pool(name="cpool", bufs=1) as cpool,
            tc.tile_pool(name="rt", bufs=2) as rt,
            tc.tile_pool(name="ps_mm", bufs=4, space="PSUM") as ps_mm,
            tc.tile_pool(name="ps_tr", bufs=2, space="PSUM") as ps_tr,
            tc.tile_pool(name="ps_sm", bufs=2, space="PSUM") as ps_sm,
        ):
            # ---- one-time prep -------------------------------------------
            ident = consts.tile([128, 128], F32)
            make_identity(nc, ident[:])
            b2_sb = consts.tile([L, D], F32)
            nc.sync.dma_start(b2_sb[:], b2[:])

            # two-step routing (matches reference fp path more closely):
            # xa = x @ P_w.T, then z = xa @ Ucat.T
            u_sb = consts.tile([D_ADDR, 3 * L], F32)
            pwt_sb = consts.tile([128, KS, D_ADDR], F32)
            nc.sync.dma_start(u_sb[:], ucat_t[:])
            nc.sync.dma_start(pwt_sb[:], p_w[:])

            # ---- persistent per-supertile tiles --------------------------
            for st in range(n_st):
                t0 = st * st_tok
                # x^T supertile [128, KS, st_tok], f32r view for the big GEMMs
                xt_sb = xpool.tile([128, KS, st_tok], F32R, name="xt_sb")
                nc.sync.dma_start(
                    xt_sb[:],
                    xt[:, t0:t0 + st_tok]
                    .rearrange("(ks kp) t -> kp ks t", kp=128)
                    .bitcast(F32R),
                )
                xt_f32 = xt_sb[:].bitcast(F32)

                # ---- routing: z = x @ A.T, then dense top-3 combine ------
                z_sb = cpool.tile([128, n_tt, 3 * L], F32, name="z_sb")
                for t in range(n_tt):
                    ps_xa = ps_sm.tile([128, D_ADDR], F32, name="ps_xa", tag="sm")
                    for ks in range(KS):
                        nc.tensor.matmul(
                            ps_xa[:],
                            xt_f32[:, ks, t * 128:(t + 1) * 128],
                            pwt_sb[:, ks, :],
                            start=(ks == 0), stop=(ks == KS - 1),
                        )
                    xa_sb = rt.tile([128, D_ADDR], F32, name="xa_sb")
                    nc.vector.tensor_copy(xa_sb[:], ps_xa[:])
                    ps_xat = ps_tr.tile([D_ADDR, 128], F32, name="ps_xat", tag="tr")
                    nc.tensor.transpose(ps_xat[:], xa_sb[:], ident[:])
                    xat_sb = rt.tile([D_ADDR, 128], F32, name="xat_sb")
                    nc.vector.tensor_copy(xat_sb[:], ps_xat[:])
                    ps_z = ps_sm.tile([128, 3 * L], F32, name="ps_z", tag="sm")
                    nc.tensor.matmul(ps_z[:], xat_sb[:], u_sb[:],
                                     start=True, stop=True)
                    nc.vector.tensor_copy(z_sb[:, t, :], ps_z[:])

                c_sb = cpool.tile([128, n_tt, 3 * L], F32, name="c_sb")
                sh12 = [128, n_tt, L]
                for j in range(3):
                    zj = z_sb[:, :, j * L:(j + 1) * L]
                    cj = c_sb[:, :, j * L:(j + 1) * L]
                    m1 = rt.tile([128, n_tt, 1], F32, name="m1")
                    m2 = rt.tile([128, n_tt, 1], F32, name="m2")
                    m3 = rt.tile([128, n_tt, 1], F32, name="m3")
                    msk = rt.tile(sh12, F32, name="msk")
                    z1 = rt.tile(sh12, F32, name="z1")
                    z2 = rt.tile(sh12, F32, name="z2")
                    te = rt.tile(sh12, F32, name="te")
                    nc.vector.reduce_max(m1[:, :, 0], zj, axis=mybir.AxisListType.X)
                    nc.vector.tensor_tensor(msk[:], zj, m1.to_broadcast(sh12),
                                            mybir.AluOpType.is_equal)
                    nc.vector.scalar_tensor_tensor(z1[:], msk[:], NEG_BIG, zj,
                                                   mybir.AluOpType.mult,
                                                   mybir.AluOpType.add)
                    nc.vector.reduce_max(m2[:, :, 0], z1[:], axis=mybir.AxisListType.X)
                    nc.vector.tensor_tensor(msk[:], z1[:], m2.to_broadcast(sh12),
                                            mybir.AluOpType.is_equal)
                    nc.vector.scalar_tensor_tensor(z2[:], msk[:], NEG_BIG, z1[:],
                                                   mybir.AluOpType.mult,
                                                   mybir.AluOpType.add)
                    nc.vector.reduce_max(m3[:, :, 0], z2[:], axis=mybir.AxisListType.X)
                    # denom = 1 + exp((m2-m1)*s) + exp((m3-m1)*s); rec = 1/denom
                    e2 = rt.tile([128, n_tt, 1], F32, name="e2")
                    e3 = rt.tile([128, n_tt, 1], F32, name="e3")
                    den = rt.tile([128, n_tt, 1], F32, name="den")
                    rec = rt.tile([128, n_tt, 1], F32, name="rec")
                    nc.vector.tensor_sub(e2[:], m2[:], m1[:])
                    nc.vector.tensor_sub(e3[:], m3[:], m1[:])
                    nc.scalar.activation(e2[:], e2[:], mybir.ActivationFunctionType.Exp,
                                         scale=TAU_SCALE)
                    nc.scalar.activation(e3[:], e3[:], mybir.ActivationFunctionType.Exp,
                                         scale=TAU_SCALE)
                    nc.vector.tensor_add(den[:], e2[:], e3[:])
                    nc.vector.tensor_scalar_add(den[:], den[:], 1.0)
                    nc.vector.reciprocal(rec[:], den[:])
                    # C = (z >= m3) * exp((z-m1)*s) * rec
                    nc.vector.tensor_sub(te[:], zj, m1.to_broadcast(sh12))
                    nc.scalar.activation(te[:], te[:], mybir.ActivationFunctionType.Exp,
                                         scale=TAU_SCALE)
                    nc.vector.tensor_tensor(msk[:], zj, m3.to_broadcast(sh12),
                                            mybir.AluOpType.is_ge)
                    nc.vector.tensor_tensor(te[:], te[:], msk[:], mybir.AluOpType.mult)
                    nc.vector.tensor_tensor(cj, te[:], rec.to_broadcast(sh12),
                                            mybir.AluOpType.mult)

                # ---- layer 1: h = gelu(sum_e C1[:,e] * (x @ W1[e].T)) ----
                h_acc = hpool.tile([128, n_tt, D], F32, name="h_acc")
                for e in range(L):
                    for half in range(2):
                        w_sb = wpool.tile([128, KS, 512], F32R, name="w_sb", tag="w")
                        nc.sync.dma_start(w_sb[:], w1t[e, half].bitcast(F32R))
                        for t in range(n_tt):
                            ps = ps_mm.tile([128, 512], F32, name="ps", tag="mm")
                            for ks in range(KS):
                                nc.tensor.matmul(
                                    ps[:],
                                    xt_sb[:, ks, t * 128:(t + 1) * 128],
                                    w_sb[:, ks, :],
                                    start=(ks == 0), stop=(ks == KS - 1),
                                )
                            hs = h_acc[:, t, half * 512:(half + 1) * 512]
                            ce = c_sb[:, t, e:e + 1]
                            if e == 0:
                                nc.vector.tensor_scalar(hs, ps[:], ce, None,
                                                        mybir.AluOpType.mult)
                            else:
                                nc.vector.scalar_tensor_tensor(
                                    hs, ps[:], ce, hs,
                                    mybir.AluOpType.mult, mybir.AluOpType.add)

                # gelu in place (exact erf-based LUT)
                for t in range(n_tt):
                    nc.scalar.activation(h_acc[:, t, :], h_acc[:, t, :], gelu_fn)

                # ---- transpose h -> ht [128, KS, st_tok] f32r ------------
                ht_sb = hpool.tile([128, KS, st_tok], F32R, name="ht_sb")
                for t in range(n_tt):
                    for hs in range(KS):
                        ps_t = ps_tr.tile([128, 128], F32, name="ps_t", tag="tr")
                        nc.tensor.transpose(
                            ps_t[:], h_acc[:, t, hs * 128:(hs + 1) * 128], ident[:])
                        nc.vector.tensor_copy(ht_sb[:, hs, t * 128:(t + 1) * 128], ps_t[:])

                # ---- layer 2 init: y = C3 @ b2 ---------------------------
                y_acc = hpool.tile([128, n_tt, D], F32, name="y_acc")
                for t in range(n_tt):
                    ps_c = ps_sm.tile([L, 128], F32, name="ps_c", tag="sm")
                    nc.tensor.transpose(ps_c[:], c_sb[:, t, 2 * L:3 * L], ident[:])
                    c3t = rt.tile([L, 128], F32, name="c3t")
                    nc.vector.tensor_copy(c3t[:], ps_c[:])
                    for half in range(2):
                        ps_b = ps_mm.tile([128, 512], F32, name="ps_b", tag="mm")
                        nc.tensor.matmul(ps_b[:], c3t[:],
                                         b2_sb[:, half * 512:(half + 1) * 512],
                                         start=True, stop=True)
                        nc.vector.tensor_copy(y_acc[:, t, half * 512:(half + 1) * 512],
                                              ps_b[:])

                # ---- layer 2: y += sum_e C2[:,e] * (h @ W2[e].T) ---------
                for e in range(L):
                    for half in range(2):
                        w_sb = wpool.tile([128, KS, 512], F32R, name="w_sb", tag="w")
                        nc.sync.dma_start(w_sb[:], w2t[e, half].bitcast(F32R))
                        for t in range(n_tt):
                            ps = ps_mm.tile([128, 512], F32, name="ps", tag="mm")
                            for ks in range(KS):
                                nc.tensor.matmul(
                                    ps[:],
                                    ht_sb[:, ks, t * 128:(t + 1) * 128],
                                    w_sb[:, ks, :],
                                    start=(ks == 0), stop=(ks == KS - 1),
                                )
                            ys = y_acc[:, t, half * 512:(half + 1) * 512]
                            ce = c_sb[:, t, L + e:L + e + 1]
                            nc.vector.scalar_tensor_tensor(
                                ys, ps[:], ce, ys,
                                mybir.AluOpType.mult, mybir.AluOpType.add)

                # ---- store y ---------------------------------------------
                nc.sync.dma_start(
                    y[t0:t0 + st_tok, :].rearrange("(tt p) d -> p tt d", p=128),
                    y_acc[:],
                )
    nc.finalize()
    return nc


_NC_CACHE = {}


def _get_nc(t_c):
    if t_c not in _NC_CACHE:
        _NC_CACHE[t_c] = build(t_c)
    return _NC_CACHE[t_c]


def w_chunks(W):
    """[L, out, in] -> [L, half, kp, ks, out512] contiguous 2MB DMA chunks."""
    return np.ascontiguousarray(
        W.reshape(L, 2, 512, KS, 128).transpose(0, 1, 4, 3, 2))


def pw_tiles(P_w):
    """[d_addr, D] -> P_w.T tiled as [kp, ks, d_addr]."""
    return np.ascontiguousarray(P_w.T.reshape(KS, 128, D_ADDR).transpose(1, 0, 2))


def _kernel_run(x, P_w, U1, U2, U3, W1, W2, b2):
    B, T, _ = x.shape
    n_tok = B * T
    t_c = n_tok // N_CORES
    xf = np.ascontiguousarray(x.reshape(n_tok, D))

    ucat_t = np.ascontiguousarray(np.concatenate([U1, U2, U3], axis=0).T)
    w1t = w_chunks(W1)
    w2t = w_chunks(W2)

    nc = _get_nc(t_c)
    in_maps = []
    for c in range(N_CORES):
        xt_c = np.ascontiguousarray(xf[c * t_c:(c + 1) * t_c].T)
        in_maps.append({
            "xt": xt_c,
            "ucat_t": ucat_t,
            "p_w": pw_tiles(P_w),
            "w1t": w1t,
            "w2t": w2t,
            "b2": np.ascontiguousarray(b2),
        })
    res = run_bass_kernel_spmd(nc, in_maps, core_ids=list(range(N_CORES)))
    y = np.concatenate([res.results[c]["y"] for c in range(N_CORES)], axis=0)
    return y.reshape(B, T, D)


def _subproc_main(tmpdir):
    import os
    ins = dict(np.load(os.path.join(tmpdir, "ins.npz")))
    y = _kernel_run(**ins)
    np.save(os.path.join(tmpdir, "y.npy"), y)


def kernel(x, P_w, U1, U2, U3, W1, W2, b2):
    """Rare transient NRT_EXEC_UNIT_UNRECOVERABLE crashes poison the whole
    process's PJRT client; retry in a fresh subprocess if the in-process
    attempt fails."""
    ins = dict(x=x, P_w=P_w, U1=U1, U2=U2, U3=U3, W1=W1, W2=W2, b2=b2)
    try:
        return _kernel_run(**ins)
    except Exception:
        import os, subprocess, sys, tempfile, traceback
        traceback.print_exc()
        last = None
        for attempt in range(2):
            tmpdir = tempfile.mkdtemp()
            np.savez(os.path.join(tmpdir, "ins.npz"), **ins)
            code = (f"import sys; sys.path.insert(0, {os.path.dirname(os.path.abspath(__file__))!r}); "
                    f"import kernel; kernel._subproc_main({tmpdir!r})")
            r = subprocess.run([sys.executable, "-c", code])
            ypath = os.path.join(tmpdir, "y.npy")
            if r.returncode == 0 and os.path.exists(ypath):
                return np.load(ypath)
            last = r.returncode
        raise RuntimeError(f"kernel subprocess retries failed (rc={last})")
